# revision 1
# baseline (speedup 1.0000x reference)
"""2-layer GCN (PyG GCNConv x2, relu between, bias=False) on 8 trn2
NeuronCores via Bass/Tile. Self-contained grading entry: kernel(**inputs).

See docstring of _kernel_impl below for the design.
"""
import sys
for _p in ("/opt/trn_rl_repo",):
    if _p not in sys.path:
        sys.path.insert(0, _p)


import numpy as np

import concourse.bass as bass
import concourse.bacc as bacc
import concourse.mybir as mybir
import concourse.tile as tile
from concourse import library_config
from concourse.bass_utils import run_bass_kernel_spmd

P = 128
D = 64          # feature dim of H1/H2/out
NCORES = 8
CHUNK = 32768   # max rows addressable by int16 idx
GTILES = 8      # dst-tiles per gather group
MB = 8          # mask batch (blocks per DVE mask build)

F32 = mybir.dt.float32
I16 = mybir.dt.int16
BATCH_MASKS = True
MAXI = 1024      # max idxs per dma_gather call (Q7 scratch limit)
NUM_Q = 4        # SWDGE queues to round-robin gathers over
SCRATCH = 16384  # dynamic_dma_scratch_size


def _ceil(a, b):
    return -(-a // b)


def perm_pos(n):
    """Storage row of node n in the H1 gather table (block-permuted so the
    H-prep SBUF tile [128, 4*64] DMAs out contiguously)."""
    s, r = n // 512, n % 512
    return s * 512 + (r % 128) * 4 + r // 128


class Structure:
    """Compile-time (cross-core shared) layout of one layer's aggregation."""

    def __init__(self, NT, n_chunks, seg_pad):
        self.NT = NT
        self.n_chunks = n_chunks
        self.seg_pad = seg_pad              # [NT, n_chunks] multiples of 128
        self.run_max = int(seg_pad.max()) // P          # longest run (blocks)
        self.nbt_max = int(seg_pad.sum(axis=1).max()) // P  # blocks per tile
        self.groups = []                    # list of dicts
        order = []                          # (t, c) stream order
        for g0 in range(0, NT, GTILES):
            tiles = list(range(g0, min(g0 + GTILES, NT)))
            for c in range(n_chunks):
                for t in tiles:
                    order.append((t, c))
        base = np.zeros((NT, n_chunks), dtype=np.int64)
        off = 0
        for (t, c) in order:
            base[t, c] = off
            off += seg_pad[t, c]
        self.base = base
        self.total = off                    # padded edge stream length
        self.nblocks = off // P
        for g0 in range(0, NT, GTILES):
            tiles = list(range(g0, min(g0 + GTILES, NT)))
            calls = []
            for c in range(n_chunks):
                n_gc = int(seg_pad[tiles[0]:tiles[-1] + 1, c].sum())
                if n_gc:
                    calls.append(dict(
                        c=c,
                        num=n_gc,
                        icol=int(base[tiles[0], c]) // 16,
                        blk=int(base[tiles[0], c]) // P,
                    ))
            gb0 = int(base[tiles[0], 0]) // P
            gb1 = gb0 + sum(cl["num"] for cl in calls) // P
            truns = {
                t: [
                    (int(base[t, c]) // P, int(seg_pad[t, c]) // P)
                    for c in range(n_chunks)
                    if seg_pad[t, c]
                ]
                for t in tiles
            }
            self.groups.append(
                dict(tiles=tiles, calls=calls, gb0=gb0, gb1=gb1, truns=truns)
            )


def build_layer_struct(core, t, c, dl, i16, NT, n_chunks):
    """Per-(core,tile,chunk) grouping of the edge stream -> Structure +
    per-core idx (wrapped int16) and dst_local (f32 [128, nblocks]) arrays."""
    n_ch = n_chunks
    key = ((core.astype(np.int64) * NT + t) * n_ch + c)
    counts = np.bincount(key, minlength=NCORES * NT * n_ch)
    counts3 = counts.reshape(NCORES, NT, n_ch)
    seg_len = counts3.max(axis=0)
    seg_pad = _ceil(seg_len, P) * P
    st = Structure(NT, n_ch, seg_pad)

    order = np.argsort(key, kind="stable")
    key_s = key[order]
    starts = np.concatenate([[0], np.cumsum(counts)[:-1]])
    rank_s = np.arange(len(key)) - starts[key_s]

    base_flat = st.base  # [NT, n_ch]
    pos_s = (
        base_flat[t[order], c[order]]
        + rank_s
        + core[order].astype(np.int64) * st.total
    )
    IDX = np.zeros(NCORES * st.total, dtype=np.int16)
    DL = np.full(NCORES * st.total, -1.0, dtype=np.float32)
    IDX[pos_s] = i16[order]
    DL[pos_s] = dl[order].astype(np.float32)
    IDX = IDX.reshape(NCORES, st.total)
    DL = DL.reshape(NCORES, st.total)

    # wrapped idx layout: idx j of the stream at [j%16, j//16], replicated x8
    wr = IDX.reshape(NCORES, st.total // 16, 16).transpose(0, 2, 1)
    idx_wrapped = np.tile(wr, (1, 8, 1))            # [NCORES, 128, total/16]
    dl_arr = DL.reshape(NCORES, st.nblocks, P).transpose(0, 2, 1).copy()
    return st, idx_wrapped, dl_arr


def host_prep(x, edge_index, W1, W2):
    N = x.shape[0]
    IN = x.shape[1]
    assert N % NCORES == 0
    SHARD = N // NCORES
    NT = _ceil(SHARD, P)
    SHARD_P = NT * P
    N_PAD = _ceil(N, 512) * 512

    src = np.asarray(edge_index[0], dtype=np.int64)
    dst = np.asarray(edge_index[1], dtype=np.int64)
    loop = np.arange(N, dtype=np.int64)
    src = np.concatenate([src, loop])
    dst = np.concatenate([dst, loop])

    deg = np.bincount(dst, minlength=N).astype(np.float32)
    dinv = (1.0 / np.sqrt(deg)).astype(np.float32)

    core = (dst // SHARD).astype(np.int32)
    l = (dst - core.astype(np.int64) * SHARD)
    t = (l >> 7).astype(np.int32)
    dl = (l & 127).astype(np.int32)

    # ---- L1: gather from H1s (block-permuted rows of [N_PAD, 64])
    pp = perm_pos(src)
    c1 = (pp >> 15).astype(np.int32)
    i1 = (pp & (CHUNK - 1)).astype(np.int16)
    n_ch1 = _ceil(N_PAD, CHUNK)
    st1, idx1, dl1 = build_layer_struct(core, t, c1, dl, i1, NT, n_ch1)

    # ---- L2: gather from AG buffers. Last chunk kept small: it is the
    # only AG on the critical path (L2 waits for all chunks; earlier ones
    # hide under L1 compute).
    if NT <= 4:
        ag_tiles = np.array_split(np.arange(NT), min(4, NT))
    else:
        last = max(1, NT // 12)
        base = NT - last
        ag_tiles = np.array_split(np.arange(base), 3) + [np.arange(base, NT)]
    n_ag = len(ag_tiles)
    tile2ag = np.zeros(NT, dtype=np.int64)
    ag_rows = []
    ag_row_start = np.zeros(NT, dtype=np.int64)  # local row offset of tile
    for k, ts in enumerate(ag_tiles):
        tile2ag[ts] = k
        ag_rows.append(len(ts) * P)
        for j, tt in enumerate(ts):
            ag_row_start[tt] = j * P
    ag_rows = np.array(ag_rows)
    assert (ag_rows * NCORES <= CHUNK).all(), "AG buffer exceeds int16 range"

    r2 = src // SHARD
    l2 = src - r2 * SHARD
    ts2 = (l2 >> 7).astype(np.int64)
    c2 = tile2ag[ts2].astype(np.int32)
    i2 = (r2 * ag_rows[c2] + ag_row_start[ts2] + (l2 & 127)).astype(np.int16)
    st2, idx2, dl2 = build_layer_struct(core, t, c2, dl, i2, NT, n_ag)

    # ---- per-core inputs
    XT = np.zeros((IN, N_PAD), dtype=np.float32)
    XT[:, :N] = np.asarray(x, dtype=np.float32).T
    dinv_col = np.ones((P, N_PAD // P), dtype=np.float32)
    dinv_col[:, : N // P] = dinv[: (N // P) * P].reshape(-1, P).T
    if N % P:
        rem = dinv[(N // P) * P:]
        dinv_col[: len(rem), N // P] = rem

    RW = max(st1.run_max, st2.run_max)
    iota_rep = np.tile(np.arange(P, dtype=np.float32), (P, RW))
    ident = np.eye(P, dtype=np.float32)
    W1h = np.asarray(W1, dtype=np.float32)
    W2h = np.asarray(W2, dtype=np.float32)

    in_maps = []
    for cc in range(NCORES):
        drow = np.ones((D, SHARD_P), dtype=np.float32)
        seg = dinv[cc * SHARD:(cc + 1) * SHARD]
        drow[:, : len(seg)] = seg[None, :]
        in_maps.append({
            "XT": XT, "W1": W1h, "W2": W2h, "ident": ident,
            "iota_rep": iota_rep, "dinv_col": dinv_col,
            "dinv_row": drow,
            "idx1": idx1[cc], "dl1": dl1[cc],
            "idx2": idx2[cc], "dl2": dl2[cc],
        })

    meta = dict(N=N, IN=IN, SHARD=SHARD, NT=NT, SHARD_P=SHARD_P,
                N_PAD=N_PAD, st1=st1, st2=st2, RW=RW,
                ag_rows=[int(r) for r in ag_rows],
                ag_tiles=[list(map(int, ts)) for ts in ag_tiles])
    return in_maps, meta


def emit_agg_layer(nc, pool, psum_pool, st, chunks, idx_dram, dl_dram,
                   iota_sb, post_tile, level=4):
    """Emit one aggregation layer.
    chunks: list of DRAM APs (gather sources per chunk index)
    post_tile(t, ps): consume the accumulated psum [64, 128] of dst tile t.
    level: 1=gathers only, 2=+masks, 3=+matmuls, 4=+post_tile
    """
    for grp in st.groups:
        gb0, gb1 = grp["gb0"], grp["gb1"]
        nbg = gb1 - gb0
        if nbg == 0:
            continue
        scols = sum(cl["num"] for cl in grp["calls"]) // 16
        icol0 = grp["calls"][0]["icol"]
        idx_sb = pool.tile([P, scols], I16, tag="idx")
        nc.sync.dma_start(idx_sb[:], idx_dram[:, icol0:icol0 + scols])
        dl_sb = pool.tile([P, nbg], F32, tag="dl")
        nc.sync.dma_start(dl_sb[:], dl_dram[:, gb0:gb1])
        msg = pool.tile([P, nbg, D], F32, tag="msg")
        for cl in grp["calls"]:
            for o in range(0, cl["num"], MAXI):
                n = min(MAXI, cl["num"] - o)
                b0 = (cl["blk"] - gb0) + o // P
                ic = (cl["icol"] - icol0) + o // 16
                nc.gpsimd.dma_gather(
                    msg[:, b0:b0 + n // P, :], chunks[cl["c"]],
                    idx_sb[:, ic: ic + n // 16], n, n, D,
                    queue_num=emit_agg_layer._qn % NUM_Q)
                emit_agg_layer._qn += 1
        if level < 2:
            continue
        for t in grp["tiles"]:
            runs = grp["truns"][t]
            nblk = sum(nb for _, nb in runs)
            if nblk == 0:
                continue
            mask_t = pool.tile([P, st.nbt_max * P], F32, tag="mask")
            off = 0
            for (bo, nb) in runs:
                lb = bo - gb0
                if BATCH_MASKS:
                    nc.vector.tensor_tensor(
                        out=mask_t[:, off * P:(off + nb) * P]
                        .rearrange("p (w d) -> p w d", d=P),
                        in0=iota_sb[:, : nb * P]
                        .rearrange("p (w d) -> p w d", d=P),
                        in1=dl_sb[:, lb:lb + nb].to_broadcast([P, nb, P]),
                        op=mybir.AluOpType.is_equal)
                else:
                    for j in range(nb):
                        nc.vector.tensor_scalar(
                            out=mask_t[:, (off + j) * P:(off + j + 1) * P],
                            in0=iota_sb[:, :P],
                            scalar1=dl_sb[:, lb + j:lb + j + 1],
                            scalar2=None,
                            op0=mybir.AluOpType.is_equal)
                off += nb
            if level < 3:
                continue
            ps = psum_pool.tile([D, P], F32, tag="agg")
            k = 0
            for (bo, nb) in runs:
                for j in range(nb):
                    nc.tensor.matmul(
                        out=ps[:], lhsT=msg[:, bo - gb0 + j, :],
                        rhs=mask_t[:, k * P:(k + 1) * P],
                        start=(k == 0), stop=(k == nblk - 1))
                    k += 1
            if level < 4:
                continue
            post_tile(t, ps)


def build_nc(meta, debug="full"):
    NT, SHARD_P, N_PAD = meta["NT"], meta["SHARD_P"], meta["N_PAD"]
    IN = meta["IN"]
    st1, st2 = meta["st1"], meta["st2"]
    n_ag = len(meta["ag_rows"])

    emit_agg_layer._qn = 0
    nc = bacc.Bacc("TRN2", target_bir_lowering=False, debug=False,
                   num_devices=NCORES, dynamic_dma_scratch_size=SCRATCH,
                   num_swdge_queues=NUM_Q)
    XT = nc.dram_tensor("XT", [IN, N_PAD], F32, kind="ExternalInput")
    W1 = nc.dram_tensor("W1", [IN, D], F32, kind="ExternalInput")
    W2 = nc.dram_tensor("W2", [D, D], F32, kind="ExternalInput")
    ident = nc.dram_tensor("ident", [P, P], F32, kind="ExternalInput")
    RW = meta["RW"]
    iota_rep = nc.dram_tensor("iota_rep", [P, RW * P], F32,
                              kind="ExternalInput")
    dinv_col = nc.dram_tensor("dinv_col", [P, N_PAD // P], F32,
                              kind="ExternalInput")
    dinv_row = nc.dram_tensor("dinv_row", [D, SHARD_P], F32,
                              kind="ExternalInput")
    idx1 = nc.dram_tensor("idx1", [P, st1.total // 16], I16,
                          kind="ExternalInput")
    dl1 = nc.dram_tensor("dl1", [P, st1.nblocks], F32, kind="ExternalInput")
    idx2 = nc.dram_tensor("idx2", [P, st2.total // 16], I16,
                          kind="ExternalInput")
    dl2 = nc.dram_tensor("dl2", [P, st2.nblocks], F32, kind="ExternalInput")
    outT = nc.dram_tensor("outT", [D, SHARD_P], F32, kind="ExternalOutput")

    with tile.TileContext(nc) as tc:
        with (
            tc.tile_pool(name="sbuf", bufs=2) as pool,
            tc.tile_pool(name="cst", bufs=1) as cst,
            tc.tile_pool(name="psum", bufs=2, space="PSUM") as psum_pool,
        ):
            nc.gpsimd.load_library(library_config.mlp)

            W1_sb = cst.tile([IN, D], F32)
            nc.sync.dma_start(W1_sb[:], W1[:])
            W2_sb = cst.tile([D, D], F32)
            nc.sync.dma_start(W2_sb[:], W2[:])
            id_sb = cst.tile([P, P], F32)
            nc.sync.dma_start(id_sb[:], ident[:])
            iota_sb = cst.tile([P, RW * P], F32)
            nc.sync.dma_start(iota_sb[:], iota_rep[:])
            dcol_sb = cst.tile([P, N_PAD // P], F32)
            nc.sync.dma_start(dcol_sb[:], dinv_col[:])
            drow_sb = cst.tile([D, SHARD_P], F32)
            nc.sync.dma_start(drow_sb[:], dinv_row[:])

            H1s = nc.dram_tensor("H1s", [N_PAD, D], F32, kind="Internal").ap()
            H2s = nc.dram_tensor("H2s", [SHARD_P, D], F32,
                                 kind="Internal").ap()
            AGs = [nc.dram_tensor(f"AG{k}", [NCORES * r, D], F32,
                                  kind="Internal", addr_space="Shared").ap()
                   for k, r in enumerate(meta["ag_rows"])]

            # ---- P0: full H1s = dinv * (X @ W1), block-permuted rows
            for s in range(N_PAD // 512):
                xt = pool.tile([P, 512], F32, tag="xt")
                nc.sync.dma_start(xt[:], XT[:, s * 512:(s + 1) * 512])
                hp = psum_pool.tile([D, 512], F32, tag="hp")
                nc.tensor.matmul(out=hp[:], lhsT=W1_sb[:], rhs=xt[:],
                                 start=True, stop=True)
                hps = pool.tile([D, 512], F32, tag="hps")
                nc.scalar.activation(out=hps[:], in_=hp[:],
                                     func=mybir.ActivationFunctionType.Copy)
                h1r = pool.tile([P, 4, D], F32, tag="h1r")
                for q in range(4):
                    tp = psum_pool.tile([P, D], F32, tag="tp")
                    nc.tensor.transpose(out=tp[:],
                                        in_=hps[:, q * P:(q + 1) * P],
                                        identity=id_sb[:D, :D])
                    nc.vector.tensor_scalar_mul(
                        out=h1r[:, q, :], in0=tp[:],
                        scalar1=dcol_sb[:, s * 4 + q: s * 4 + q + 1])
                nc.sync.dma_start(
                    H1s[s * 512:(s + 1) * 512, :]
                    .rearrange("(p q) d -> p (q d)", p=P),
                    h1r[:].rearrange("p q d -> p (q d)"))

            # ---- P1: layer-1 aggregation + H2s tiles + chunked AllGather
            chunk1 = []
            for c in range(st1.n_chunks):
                b0 = c * CHUNK
                chunk1.append(H1s[b0:min(b0 + CHUNK, N_PAD), :])

            ag_last_tile = [ts[-1] for ts in meta["ag_tiles"]]
            if debug in ("p1", "timing"):
                ag_last_tile = []

            def post_l1(t, ps):
                srow = drow_sb[:, t * P:(t + 1) * P]
                o1 = pool.tile([D, P], F32, tag="o1")
                nc.vector.tensor_tensor(out=o1[:], in0=ps[:], in1=srow,
                                        op=mybir.AluOpType.mult)
                nc.scalar.activation(out=o1[:], in_=o1[:],
                                     func=mybir.ActivationFunctionType.Relu)
                h2p = psum_pool.tile([D, P], F32, tag="h2")
                nc.tensor.matmul(out=h2p[:], lhsT=W2_sb[:], rhs=o1[:],
                                 start=True, stop=True)
                h2s = pool.tile([D, P], F32, tag="h2s")
                nc.vector.tensor_tensor(out=h2s[:], in0=h2p[:], in1=srow,
                                        op=mybir.AluOpType.mult)
                tp = psum_pool.tile([P, D], F32, tag="tp")
                nc.tensor.transpose(out=tp[:], in_=h2s[:],
                                    identity=id_sb[:D, :D])
                h2r = pool.tile([P, D], F32, tag="h2r")
                nc.scalar.activation(out=h2r[:], in_=tp[:],
                                     func=mybir.ActivationFunctionType.Copy)
                nc.sync.dma_start(H2s[t * P:(t + 1) * P, :], h2r[:])
                if t in ag_last_tile:
                    k = ag_last_tile.index(t)
                    ts = meta["ag_tiles"][k]
                    nc.gpsimd.collective_compute(
                        "AllGather", mybir.AluOpType.bypass,
                        replica_groups=[list(range(NCORES))],
                        ins=[H2s[ts[0] * P: (ts[-1] + 1) * P, :]],
                        outs=[AGs[k][:]])

            if debug != "p0":
                level = {"g": 1, "gm": 2, "gmm": 3}.get(debug, 4)
                emit_agg_layer(nc, pool, psum_pool, st1, chunk1, idx1, dl1,
                               iota_sb, post_l1, level=level)

            if debug in ("full", "timing"):
                # ---- P2: layer-2 aggregation -> outT
                if debug == "timing":
                    # collective-free variant for TimelineSim: same traffic
                    # pattern, reads H1s instead of AG buffers
                    chunk2 = [H1s[0:NCORES * r, :] for r in meta["ag_rows"]]
                else:
                    chunk2 = [ag[:] for ag in AGs]

                def post_l2(t, ps):
                    srow = drow_sb[:, t * P:(t + 1) * P]
                    o2 = pool.tile([D, P], F32, tag="o2")
                    nc.vector.tensor_tensor(out=o2[:], in0=ps[:], in1=srow,
                                            op=mybir.AluOpType.mult)
                    nc.sync.dma_start(outT[:, t * P:(t + 1) * P], o2[:])

                emit_agg_layer(nc, pool, psum_pool, st2, chunk2, idx2, dl2,
                               iota_sb, post_l2)

    nc.compile()
    return nc


def _kernel_run(x, edge_index, W1, W2, trace=False, trace_kwargs=None):
    in_maps, meta = host_prep(x, edge_index, W1, W2)
    nc = build_nc(meta)
    res = run_bass_kernel_spmd(
        nc, in_maps, core_ids=list(range(NCORES)),
        trace=trace, **(trace_kwargs or {}))
    N, SHARD = meta["N"], meta["SHARD"]
    out = np.empty((N, D), dtype=np.float32)
    for c in range(NCORES):
        out[c * SHARD:(c + 1) * SHARD] = res.results[c]["outT"].T[:SHARD]
    return out, res


def kernel(x, edge_index, W1, W2):
    """Grading entry: full (unsharded) inputs -> full [N, 64] output."""
    out, _res = _kernel_run(np.asarray(x), np.asarray(edge_index),
                            np.asarray(W1), np.asarray(W2))
    return out

